# revision 2
# baseline (speedup 1.0000x reference)
"""CrossNet kernel for Trainium2 (8 NeuronCores, pure data parallel).

Math (reference: x_{l+1} = x0*(x_l.w_l) + x_l + b_l, unrolled; bias==0 in
the shipped problem, the general path folds it):
    A_i = 1 + x . w_i            (per-row, i = 0..2)
    T3  = (A_0*A_1 + beta1)*A_2 + beta2
    out = x * T3 (+ b0+b1+b2)

Layout: per core x is [2048, 1024] viewed as [128, 16, 1024] — partition p
holds rows 16p..16p+15 (row r = 16p + t). All dot-product compute in bf16
(inputs ~N(0,1); measured rel err ~2.4e-3, gate is 2e-2).

Schedule (engine assignment from per-op microbenchmarks):
  - Loads: SWDGE casting DMAs fp32->bf16 on the gpsimd queue (measured at
    full HBM rate ~380 GB/s) — removes the cast pass entirely. Uneven
    chunks [2,4,4,3,2,1] slots: small first chunk starts compute sooner,
    small last chunk shrinks the drain tail.
  - DVE: per chunk, one broadcast bf16 tensor_tensor multiply for dot 2
    (2x perf mode, ~0.52 ns/elem) issued first so ScalarE can start
    reducing, then per 128-row tile two fused scalar_tensor_tensor dots
    (bf16, fp32 accumulator, 1226 ns each) for dots 0,1, then tiny T3 ops.
  - ScalarE: per tile, activation-accumulate reduce of the dot-2 product,
    and the final out = x*T3 via activation Copy with per-partition scale
    (bf16 in -> fp32 out, 1187 ns); dispatches the chunk stores on the
    ACT HWDGE ring.
  - fused-TTR(bf16) and bf16->bf16 segmented reduce are avoided: the
    former locks up the device, the latter runs at 1x and breaks numerics.

Roofline: 16.8 MB HBM traffic/core at ~360 GB/s ~= 47 us; DVE busy ~54 us;
measured ~70-80 us vs 115 us baseline.
"""

import numpy as np
import ml_dtypes

import concourse.bacc as bacc
import concourse.mybir as mybir
import concourse.tile as tile
from concourse.bass_utils import run_bass_kernel_spmd

BATCH, DIM, LAYERS = 16384, 1024, 3
NCORES = 8
ROWS = BATCH // NCORES   # 2048 rows per core
P = 128                  # SBUF partitions
SLOTS = ROWS // P        # 16 row-slots per partition
CHUNKS = [2, 4, 4, 4, 2]
assert sum(CHUNKS) == SLOTS

F32 = mybir.dt.float32
BF16 = mybir.dt.bfloat16

mult = mybir.AluOpType.mult
add = mybir.AluOpType.add
Copy = mybir.ActivationFunctionType.Copy


def _build(with_bias: bool, beta1: float, beta2: float):
    nc = bacc.Bacc("TRN2", target_bir_lowering=False, debug=False)

    x_d = nc.dram_tensor("x", [P, SLOTS * DIM], F32, kind="ExternalInput").ap()
    w_d = nc.dram_tensor("w", [P, LAYERS * DIM], BF16, kind="ExternalInput").ap()
    if with_bias:
        b3_d = nc.dram_tensor("b3", [P, DIM], BF16, kind="ExternalInput").ap()
    out_d = nc.dram_tensor("out", [P, SLOTS * DIM], F32, kind="ExternalOutput").ap()

    with tile.TileContext(nc) as tc, \
            tc.tile_pool(name="main", bufs=1) as pool, \
            tc.tile_pool(name="outs", bufs=4) as opool, \
            tc.tile_pool(name="p2", bufs=3) as p2pool:
        wb = pool.tile([P, LAYERS, DIM], BF16, name="wb", tag="wb")
        nc.sync.dma_start(wb[:], w_d.rearrange("p (l d) -> p l d", l=LAYERS))
        if with_bias:
            b3 = pool.tile([P, DIM], BF16, name="b3", tag="b3")
            nc.sync.dma_start(b3[:], b3_d[:])

        xb = pool.tile([P, SLOTS, DIM], BF16, name="xb", tag="xb")
        scr = pool.tile([P, DIM], BF16, name="scr", tag="scr")
        sc_scr = pool.tile([P, DIM], BF16, name="sc_scr", tag="sc_scr")
        A = pool.tile([P, LAYERS, SLOTS], F32, name="A", tag="A")
        Ap = pool.tile([P, LAYERS, SLOTS], F32, name="Ap", tag="Ap")
        t2 = pool.tile([P, SLOTS], F32, name="t2", tag="t2")
        t3 = pool.tile([P, SLOTS], F32, name="t3", tag="t3")

        xv = x_d.rearrange("p (s d) -> p s d", s=SLOTS)

        bounds = []
        s = 0
        for n in CHUNKS:
            bounds.append((s, s + n))
            s += n

        # all casting loads up-front: the SWDGE queue streams them
        # back-to-back at HBM rate
        for s0, s1 in bounds:
            nc.gpsimd.dma_start(xb[:, s0:s1, :], xv[:, s0:s1, :])

        for s0, s1 in bounds:
            cn = s1 - s0
            # dot 2 multiply first: unblocks ScalarE's reduce chain while
            # DVE still runs this chunk's fused dots
            prod2 = p2pool.tile([P, cn, DIM], BF16, name="prod2", tag="prod2")
            wb2 = wb[:, 2, :].unsqueeze(1).broadcast_to([P, cn, DIM])
            nc.vector.tensor_mul(prod2[:], xb[:, s0:s1, :], wb2)
            for j in range(cn):
                t = s0 + j
                nc.scalar.activation(
                    sc_scr[:], prod2[:, j, :], Copy,
                    accum_out=A[:, 2, t:t + 1],
                )
            # dots 0,1: fused multiply+accumulate on DVE
            for t in range(s0, s1):
                for i in range(2):
                    nc.vector.scalar_tensor_tensor(
                        scr[:], xb[:, t, :], 1.0, wb[:, i, :],
                        op0=mult, op1=mult,
                        accum_out=A[:, i, t:t + 1],
                    )
            # T3 = ((1+a0)(1+a1) + beta1)(1+a2) + beta2 over the chunk
            nc.vector.tensor_scalar_add(Ap[:, :, s0:s1], A[:, :, s0:s1], 1.0)
            nc.vector.tensor_mul(t2[:, s0:s1], Ap[:, 0, s0:s1], Ap[:, 1, s0:s1])
            if beta1 != 0.0:
                nc.vector.tensor_scalar_add(t2[:, s0:s1], t2[:, s0:s1], beta1)
            nc.vector.tensor_mul(t3[:, s0:s1], t2[:, s0:s1], Ap[:, 2, s0:s1])
            if beta2 != 0.0:
                nc.vector.tensor_scalar_add(t3[:, s0:s1], t3[:, s0:s1], beta2)

            xo = opool.tile([P, cn, DIM], F32, name="xo", tag="xo")
            for j in range(cn):
                t = s0 + j
                if with_bias:
                    # out = xb*t3 + b3 (per-row scalar t3, column vector b3)
                    nc.vector.scalar_tensor_tensor(
                        xo[:, j, :], xb[:, t, :], t3[:, t:t + 1], b3[:],
                        op0=mult, op1=add,
                    )
                else:
                    nc.scalar.activation(
                        xo[:, j, :], xb[:, t, :], Copy, scale=t3[:, t:t + 1]
                    )
            nc.scalar.dma_start(
                out_d[:, s0 * DIM:s1 * DIM], xo.rearrange("p c d -> p (c d)")
            )

    nc.compile()
    return nc


def prepare(x: np.ndarray, kernels: np.ndarray, bias: np.ndarray):
    """Build the Bass program and the per-core input maps."""
    x = np.ascontiguousarray(x, dtype=np.float32)
    kernels = np.asarray(kernels, dtype=np.float32)
    bias = np.asarray(bias, dtype=np.float32)

    beta1 = float(bias[0] @ kernels[1])
    beta2 = float((bias[0] + bias[1]) @ kernels[2])
    b3 = bias.sum(axis=0)
    with_bias = bool(np.any(b3 != 0.0))

    nc = _build(with_bias, beta1, beta2)

    w_rep = np.ascontiguousarray(np.broadcast_to(
        kernels.reshape(1, LAYERS * DIM), (P, LAYERS * DIM)
    ).astype(ml_dtypes.bfloat16))
    in_maps = []
    for c in range(NCORES):
        m = {
            "x": x[c * ROWS:(c + 1) * ROWS].reshape(P, SLOTS * DIM),
            "w": w_rep,
        }
        if with_bias:
            m["b3"] = np.ascontiguousarray(
                np.broadcast_to(b3, (P, DIM)).astype(ml_dtypes.bfloat16))
        in_maps.append(m)
    return nc, in_maps


def kernel(x: np.ndarray, kernels: np.ndarray, bias: np.ndarray) -> np.ndarray:
    nc, in_maps = prepare(x, kernels, bias)
    res = run_bass_kernel_spmd(nc, in_maps, list(range(NCORES)))
    return np.concatenate(
        [r["out"].reshape(ROWS, DIM) for r in res.results], axis=0)
